# revision 16
# baseline (speedup 1.0000x reference)
"""Causal self-attention (B=2, T=4096, C=768, NH=12) on 8 trn2 NeuronCores.

Sharding: data parallel on B (cores 0-3 -> batch 0, cores 4-7 -> batch 1),
tensor parallel on heads (3 heads per core).  Each core computes, for its
batch b and heads [h0, h0+1, h0+2]:
  qk^T = (x_b @ Wqk_slice)^T          (bf16 matmuls, W stationary)
  v    = x_b @ Wv_slice               (x^T chunks stationary -> v lands in
                                       [token, dim] layout, no transposes)
  per head: causal attention entirely in SBUF
      s^T[k,q] = k^T . q^T   (scores transposed; softmax denom via a ones
      column appended to v so one PV matmul yields both y^T and denom)
  y^T normalized via DVE fast reciprocal + PE broadcast
  z^T_partial = (y @ Wp_rows)^T       (per-core partial of output proj)
Host sums the 4 partials per batch and adds the biases' contribution
(v-bias folds exactly into the output bias since softmax weights sum to 1).

The whole kernel is one software pipeline over the 8 token chunks:
qk-projection(chunk) -> v-projection(chunk) -> attention(q-chunk) ->
normalize -> z-projection(chunk), so PE / ACT / DVE / DMA overlap
throughout instead of phase-by-phase.

x/W_qkv/q/k/v/p are bf16 (1 cyc/row matmuls, half DMA+SBUF traffic);
y and W_proj stay tf32 (f32r); all accumulation is fp32 in PSUM.
"""

import sys

for _p in ("/opt/trn_rl_repo",):
    if _p not in sys.path:
        sys.path.insert(0, _p)

import numpy as np
from contextlib import ExitStack

from concourse import bacc, tile, mybir

B, T, C = 2, 4096, 768
NH, HD = 12, 64
NCORES = 8
HPC = 3             # heads per core
QC = 512            # q chunk width (N dim of matmuls)
NQC = T // QC       # 8
KB = 128            # k block (partition dim of transposed scores)
NKB = T // KB       # 32
QKW = 2 * HPC * HD  # 384 q+k columns per core
KW = 3 * HPC * HD   # 576 qkv columns per core
VS = HD + 4         # vones block stride (8B-aligned for bf16)
f32 = mybir.dt.float32
f32r = mybir.dt.float32r
bf16 = mybir.dt.bfloat16
u16 = mybir.dt.uint16
u32 = mybir.dt.uint32


def build_program(nrep=1):
    nc = bacc.Bacc("TRN2", target_bir_lowering=False, debug=False,
                   num_devices=NCORES)
    xT = nc.dram_tensor("xT", [C, T], bf16, kind="ExternalInput").ap()
    wqkv = nc.dram_tensor("wqkv", [C, KW], bf16, kind="ExternalInput").ap()
    bqkv = nc.dram_tensor("bqkv", [QKW, 1], f32, kind="ExternalInput").ap()
    wp = nc.dram_tensor("wp", [HPC * HD, C], f32r, kind="ExternalInput").ap()
    masks = nc.dram_tensor("masks", [KB, 4 * QC], bf16,
                           kind="ExternalInput").ap()
    zT = nc.dram_tensor("zT", [C, T], f32, kind="ExternalOutput").ap()

    with tile.TileContext(nc) as tc:
        for _ in range(nrep):
            _body(tc, xT, wqkv, bqkv, wp, masks, zT)
    nc.compile()
    return nc


def _body(tc, xT, wqkv, bqkv, wp, masks, zT):
    nc = tc.nc
    Exp = mybir.ActivationFunctionType.Exp
    mult = mybir.AluOpType.mult
    add = mybir.AluOpType.add

    with ExitStack() as ctx:
        # ---------------- SBUF pools (all live for the whole pipeline) ----
        qk_pool = ctx.enter_context(tc.tile_pool(name="qk", bufs=4))
        y_pool = ctx.enter_context(tc.tile_pool(name="yt", bufs=2))
        wp_pool = ctx.enter_context(tc.tile_pool(name="wp", bufs=2))
        wq_pool = ctx.enter_context(tc.tile_pool(name="wq", bufs=6))
        mask_pool = ctx.enter_context(tc.tile_pool(name="mask", bufs=1))
        const_pool = ctx.enter_context(tc.tile_pool(name="const", bufs=1))
        bias_pool = ctx.enter_context(tc.tile_pool(name="bias", bufs=3))
        vones_pool = ctx.enter_context(tc.tile_pool(name="vones", bufs=1))
        xk_pool = ctx.enter_context(tc.tile_pool(name="xk", bufs=6))
        p_pool = ctx.enter_context(tc.tile_pool(name="p", bufs=3))
        z_pool = ctx.enter_context(tc.tile_pool(name="z", bufs=1))
        d_pool = ctx.enter_context(tc.tile_pool(name="d", bufs=1))
        # PSUM pools: 4 + 2 + 2 = 8 banks
        s_ps = ctx.enter_context(tc.tile_pool(name="sps", bufs=2, space="PSUM"))
        y_ps = ctx.enter_context(tc.tile_pool(name="yps", bufs=2, space="PSUM"))
        mm_ps = ctx.enter_context(tc.tile_pool(name="mmps", bufs=2,
                                               space="PSUM"))

        # persistent tiles
        # q^T/k^T layout: A=[q_l0;q_l1] B=[k_l0;k_l1] Cqk=[q_l2;k_l2]
        # E2 = copy of k_l2 at base partition 0 (q2 and k2 must share a base)
        A = qk_pool.tile([128, T], bf16, tag="qk", name="qk")
        Bt = qk_pool.tile([128, T], bf16, tag="qk", name="qk")
        Cqk = qk_pool.tile([128, T], bf16, tag="qk", name="qk")
        E2 = qk_pool.tile([128, T], bf16, tag="qk", name="qk")
        F = y_pool.tile([128, T], f32r, tag="yt", name="yt")   # y^T l0,l1
        G = y_pool.tile([64, T], f32r, tag="yt", name="yt")    # y^T l2
        wp0 = wp_pool.tile([128, C], f32r, tag="wp", name="wp")
        wp1 = wp_pool.tile([64, C], f32r, tag="wp", name="wp")
        mask_t = mask_pool.tile([KB, 4 * QC], bf16, tag="mask", name="mask")
        ones_t = const_pool.tile([1, 64], f32r, tag="ones", name="ones")
        vones = [vones_pool.tile([128, NKB * VS], bf16, tag=f"vones{i}",
                                 name=f"vones{i}") for i in range(HPC)]

        wq_t = []
        for kt in range(6):
            w = wq_pool.tile([128, KW], bf16, tag="wq", name="wq")
            nc.sync.dma_start(out=w[:], in_=wqkv[kt * 128:(kt + 1) * 128, :])
            wq_t.append(w)
        bias_t = []
        for m in range(3):
            bt_ = bias_pool.tile([128, 1], f32, tag="bias", name="bias")
            nc.sync.dma_start(out=bt_[:], in_=bqkv[m * 128:(m + 1) * 128, :])
            bias_t.append(bt_)
        nc.sync.dma_start(out=mask_t[:], in_=masks[:])
        nc.sync.dma_start(out=wp0[:], in_=wp[0:128, :])
        nc.sync.dma_start(out=wp1[:], in_=wp[128:192, :])
        nc.gpsimd.memset(ones_t[:].bitcast(u32), 0x3F800000)  # 1.0f
        for h in range(HPC):
            nc.gpsimd.memset(vones[h][:].bitcast(u16), 0x3F80)  # 1.0bf

        qmap = [A[0:64, :], A[64:128, :], Cqk[0:64, :]]
        kmap = [Bt[0:64, :], Bt[64:128, :], E2[0:64, :]]

        def finish_chunk(item):
            qc, dsl, rrows = item
            for h in range(HPC):
                rb = mm_ps.tile([128, QC], f32, tag="mm", name="mm")
                nc.tensor.matmul(rb[0:64, :], lhsT=ones_t[:],
                                 rhs=rrows[h][:], start=True, stop=True)
                if h == 0:
                    ydst = F[0:64, dsl]
                elif h == 1:
                    ydst = F[64:128, dsl]
                else:
                    ydst = G[0:64, dsl]
                nc.vector.tensor_tensor(out=ydst, in0=ydst, in1=rb[0:64, :],
                                        op=mult)
            for mc in range(6):
                zp = mm_ps.tile([128, QC], f32, tag="mm", name="mm")
                nc.tensor.matmul(zp[:],
                                 lhsT=wp0[:, mc * 128:(mc + 1) * 128],
                                 rhs=F[:, dsl], start=True, stop=False)
                nc.tensor.matmul(zp[:],
                                 lhsT=wp1[:, mc * 128:(mc + 1) * 128],
                                 rhs=G[:, dsl], start=False, stop=True)
                zs = z_pool.tile([128, QC], f32, tag="z", name="z")
                nc.vector.tensor_copy(out=zs[:], in_=zp[:])
                nc.sync.dma_start(out=zT[mc * 128:(mc + 1) * 128, dsl],
                                  in_=zs[:])

        # ================= the pipeline over token chunks =================
        pending = []
        for tci in range(NQC):
            csl = slice(tci * QC, (tci + 1) * QC)

            # ---- q/k^T projection for this chunk (3 psum blocks) ----
            xk = []
            for kt in range(6):
                xt_ = xk_pool.tile([128, QC], bf16, tag="xk", name="xk")
                nc.sync.dma_start(
                    out=xt_[:],
                    in_=xT[kt * 128:(kt + 1) * 128, csl])
                xk.append(xt_)
            mdst = [A, Cqk, Bt]
            for m in range(3):
                dst = mdst[m]
                ps = mm_ps.tile([128, QC], f32, tag="mm", name="mm")
                for kt in range(6):
                    nc.tensor.matmul(
                        ps[:],
                        lhsT=wq_t[kt][:, m * 128:(m + 1) * 128],
                        rhs=xk[kt][:],
                        start=(kt == 0), stop=(kt == 5))
                nc.vector.tensor_scalar(dst[:, csl], ps[:], bias_t[m][:],
                                        None, add)
                if m == 1:   # k_l2 at base 0 and q_l2 at base 64 (pairing)
                    nc.vector.tensor_scalar(
                        E2[0:64, csl], ps[64:128, :],
                        bias_t[1][64:128, :], None, add)
                    nc.vector.tensor_scalar(
                        E2[64:128, csl], ps[0:64, :],
                        bias_t[1][0:64, :], None, add)

            # ---- v projection for this chunk's 4 k-blocks ----
            # x^T chunk stationary -> v lands in [token, dim] layout
            for jj in range(4):
                j = 4 * tci + jj
                vps = mm_ps.tile([128, HPC * HD], f32, tag="mm", name="mm")
                for kt in range(6):
                    nc.tensor.matmul(
                        vps[:],
                        lhsT=xk[kt][:, jj * KB:(jj + 1) * KB],
                        rhs=wq_t[kt][:, QKW:KW],
                        start=(kt == 0), stop=(kt == 5))
                for h in range(HPC):
                    nc.vector.tensor_copy(
                        out=vones[h][:, j * VS:j * VS + HD],
                        in_=vps[:, h * HD:(h + 1) * HD])

            # ---- attention for q chunk qc == tci ----
            qc = tci
            dsl = csl
            nkb = 4 * qc + 4

            def apply_masks(pt, js):
                for si, j in enumerate(js):
                    m = j - 4 * qc
                    if m >= 0:   # diagonal block: apply causal mask
                        nc.vector.tensor_tensor(
                            out=pt[:, si * QC:(si + 1) * QC],
                            in0=pt[:, si * QC:(si + 1) * QC],
                            in1=mask_t[:, m * QC:(m + 1) * QC],
                            op=mult)

            rrows = []

            def drain_y(h, yps):
                ydst = (F[0:64, dsl], F[64:128, dsl], G[0:64, dsl])[h]
                nc.vector.tensor_copy(out=ydst, in_=yps[0:64, :])
                dden = d_pool.tile([1, QC], f32, tag="dden", name="dden",
                                   bufs=3)
                nc.vector.tensor_copy(out=dden[:], in_=yps[64:65, :])
                rraw = d_pool.tile([1, QC], f32, tag="rraw", name="rraw",
                                   bufs=3)
                nc.vector.reciprocal_approx_fast(out=rraw[:], in_=dden[:])
                rrow = d_pool.tile([1, QC], f32r, tag="rrow", name="rrow",
                                   bufs=3)
                nc.vector.tensor_copy(out=rrow[:], in_=rraw[:])
                rrows.append(rrow)

            # heads 0,1: QK row-paired via partition bases 0/64
            yps01 = [y_ps.tile([65, QC], f32, tag="y", name="y")
                     for _ in range(2)]
            for jp in range(nkb // 2):
                js = (2 * jp, 2 * jp + 1)
                s01 = [s_ps.tile([128, 2 * QC], f32, tag="s", name="s")
                       for _ in range(2)]
                for si, j in enumerate(js):
                    for h in range(2):   # adjacent emission -> concurrent
                        nc.tensor.matmul(
                            s01[h][:, si * QC:(si + 1) * QC],
                            lhsT=kmap[h][:, j * KB:(j + 1) * KB],
                            rhs=qmap[h][:, dsl],
                            start=True, stop=True)
                for h in range(2):
                    pt = p_pool.tile([128, 2 * QC], bf16, tag="p", name="p")
                    nc.scalar.activation(pt[:], s01[h][:], Exp)
                    apply_masks(pt, js)
                    for si, j in enumerate(js):
                        nc.tensor.matmul(
                            yps01[h][:],
                            lhsT=vones[h][:, j * VS:j * VS + HD + 1],
                            rhs=pt[:, si * QC:(si + 1) * QC],
                            start=(j == 0), stop=(j == nkb - 1))
            for h in range(2):
                drain_y(h, yps01[h])

            # head 2: QK paired across even/odd k-blocks
            # even j: k2@E2[0:64] x q2@Cqk[0:64]; odd j: k2@Cqk[64:128]
            # x q2@E2[64:128]
            yps2 = y_ps.tile([65, QC], f32, tag="y", name="y")
            for jp in range(nkb // 2):
                js = (2 * jp, 2 * jp + 1)
                sps = s_ps.tile([128, 2 * QC], f32, tag="s", name="s")
                nc.tensor.matmul(sps[:, 0:QC],
                                 lhsT=E2[0:64, js[0] * KB:(js[0] + 1) * KB],
                                 rhs=Cqk[0:64, dsl], start=True, stop=True)
                nc.tensor.matmul(sps[:, QC:2 * QC],
                                 lhsT=Cqk[64:128,
                                          js[1] * KB:(js[1] + 1) * KB],
                                 rhs=E2[64:128, dsl], start=True, stop=True)
                pt = p_pool.tile([128, 2 * QC], bf16, tag="p", name="p")
                nc.scalar.activation(pt[:], sps[:], Exp)
                apply_masks(pt, js)
                for si, j in enumerate(js):
                    nc.tensor.matmul(
                        yps2[:],
                        lhsT=vones[2][:, j * VS:j * VS + HD + 1],
                        rhs=pt[:, si * QC:(si + 1) * QC],
                        start=(j == 0), stop=(j == nkb - 1))
            drain_y(2, yps2)

            if pending:
                finish_chunk(pending.pop(0))
            pending.append((qc, dsl, rrows))

        while pending:
            finish_chunk(pending.pop(0))


# ---------------------------------------------------------------------------
# host-side sharding / unsharding
# ---------------------------------------------------------------------------

def tf32_round(a):
    """Round fp32 array to tf32 (fp32r): RNE to 10 mantissa bits."""
    b = np.ascontiguousarray(a, dtype=np.float32).view(np.uint32).copy()
    b += 0x0FFF + ((b >> 13) & 1)
    b &= np.uint32(0xFFFFE000)
    return b.view(np.float32)


def _core_cols(h0):
    """wqkv column order per core: [q0 q1 q2 k2 k0 k1 v0 v1 v2] (local)."""
    idx = []
    for blk, l in [(0, 0), (0, 1), (0, 2), (1, 2), (1, 0), (1, 1),
                   (2, 0), (2, 1), (2, 2)]:
        g = h0 + l
        idx.append(np.arange(HD) + blk * C + g * HD)
    return np.concatenate(idx)


def make_masks():
    import ml_dtypes
    m = np.zeros((KB, 4 * QC), dtype=np.float32)
    kp = np.arange(KB)[:, None]
    qf = np.arange(QC)[None, :]
    for mi in range(4):
        m[:, mi * QC:(mi + 1) * QC] = (kp <= qf - 128 * mi)
    return m.astype(ml_dtypes.bfloat16)


def shard_inputs(x, w_attn, b_attn):
    import ml_dtypes
    x = np.ascontiguousarray(np.asarray(x, dtype=np.float32))
    w_attn = np.asarray(w_attn, dtype=np.float32)
    b_attn = np.asarray(b_attn, dtype=np.float32)
    masks = make_masks()
    in_maps = []
    for c in range(NCORES):
        b = c // 4
        h0 = HPC * (c % 4)
        cols = _core_cols(h0)
        wq = w_attn[:, cols].copy()
        bq = b_attn[cols].copy()
        # fold 1/sqrt(HD)=0.125 into the q columns (exact power of two)
        wq[:, 0:HPC * HD] *= 0.125
        bq[0:HPC * HD] *= 0.125
        in_maps.append({
            "xT": np.ascontiguousarray(x[b].T).astype(ml_dtypes.bfloat16),
            "wqkv": np.ascontiguousarray(wq).astype(ml_dtypes.bfloat16),
            "bqkv": bq[0:QKW].reshape(QKW, 1).copy(),
            "wp": None,  # filled below
            "masks": masks,
        })
    return in_maps


def fill_wp(in_maps, w_proj):
    w_proj = np.asarray(w_proj, dtype=np.float32)
    for c in range(NCORES):
        h0 = HPC * (c % 4)
        rows = np.concatenate(
            [np.arange(HD) + (h0 + l) * HD for l in range(HPC)])
        in_maps[c]["wp"] = tf32_round(np.ascontiguousarray(w_proj[rows, :]))
    return in_maps


def gather_outputs(results, b_proj, vbias_z=0.0):
    b_proj = np.asarray(b_proj, dtype=np.float32)
    y = np.zeros((B, T, C), dtype=np.float32)
    for c in range(NCORES):
        b = c // 4
        y[b] += results[c]["zT"].T
    y += (b_proj + vbias_z)[None, None, :]
    return y


_NC_CACHE = {}


def get_nc():
    if "nc" not in _NC_CACHE:
        _NC_CACHE["nc"] = build_program()
    return _NC_CACHE["nc"]


def run_spmd(in_maps, trace=False, **kw):
    from concourse.bass_utils import run_bass_kernel_spmd
    nc = get_nc()
    return run_bass_kernel_spmd(nc, in_maps, core_ids=list(range(NCORES)),
                                trace=trace, **kw)


def kernel(x, w_attn, b_attn, w_proj, b_proj):
    in_maps = shard_inputs(x, w_attn, b_attn)
    fill_wp(in_maps, w_proj)
    # One retry: a previous heavy session can leave the first device touch
    # of a fresh process reporting a stale unrecoverable exec unit; the
    # runtime resets it and the second attempt goes through.
    try:
        res = run_spmd(in_maps)
    except Exception:
        import time as _t
        _t.sleep(10)
        res = run_spmd(in_maps)
    # v-bias contribution: softmax weights sum to 1, so y += b_v exactly,
    # hence z += b_v @ w_proj -- added host-side, never touches the device.
    b_attn = np.asarray(b_attn, dtype=np.float32)
    w_proj = np.asarray(w_proj, dtype=np.float32)
    vbias_z = b_attn[2 * C:3 * C] @ w_proj
    return gather_outputs(res.results, b_proj, vbias_z)
